# revision 71
# baseline (speedup 1.0000x reference)
"""Trainium2 Bass kernel for nn_Attention_6201932775733 (sparse window attention).

Strategy (8 NeuronCores, SPMD, data-parallel over (batch, row-stripe)):
  - Core i handles batch i//4, two 16-row stripes {2*(i%4), 2*(i%4)+1}.
  - Inputs host-permuted to window-major position order, packed [128, kt, pos]
    and quantized fp8e4m3 (plus a /16-scaled copy) for DoubleRow matmuls:
    K=256 contraction in one instruction at 0.5 cycles/row.
  - Weights use a scaled residual split W ~= W8 + fp8(16*(W-W8))/16; the /16
    rides on the second input copy, so two DR matmuls accumulating in PSUM
    recover near-bf16 weight precision at half the PE cost of bf16.
  - RoPE via rotated extra projection rows; combine q' = pm*cos + pr*sin:
    two DVE muls (fused with the PSUM eviction) + one Pool add (SBUF only,
    GPSIMD cannot read PSUM).
  - Attention per window (y, head): 4 sim fp8-DR matmuls into a [128,4,256]
    PSUM tile, double-buffered (psim bufs=2) so the per-window ACT exp
    ([128,1024] -> fp8) runs back-to-back with the next window's sims.
    AV + all-ones denominator matmuls pack [av0 av1 d0 d1] per 2 head-pairs;
    one DVE reciprocal + one DVE mult normalize both (TensorTensor may read
    only ONE input from PSUM, and the DVE ISA has no divide, so recip via
    SBUF is mandatory); u stays bf16.
  - PSUM-eviction copies (pass/V/out) alternate DVE/ACT per K_EVPAT to
    balance the two eviction-capable engines (ACT also owns exp; Pool/GpSimd
    cannot read PSUM, so it only gets the SBUF-side rope adds).
  - Output projection bf16; bias injected by a K=1 f32r matmul that seeds
    PSUM; result copied to SBUF and DMA'd (DMA cannot read PSUM).
  - Single interleaved attention stream over both stripes with a pull-plan
    that drip-feeds projection/output-projection units as PE/DVE/ACT filler
    between windows; startup DMAs ordered so the first projection's operands
    (Wq + nt0 chunk) land first.
"""

import os
import numpy as np

HEADS, WIN, DH, DRR = 8, 16, 64, 32
B, C, H, W = 2, 256, 128, 128
NCORES = 8
SPOS = WIN * W          # 2048 positions per stripe
NT = 4                  # 512-wide position tiles
YW = W // WIN           # 8 y-windows per stripe
SC = 16.0               # residual weight scale

_CACHE = {}


def _build():
    import concourse.bass as bass
    import concourse.mybir as mybir
    import concourse.tile as tile
    from contextlib import ExitStack

    f32 = mybir.dt.float32
    f32r = mybir.dt.float32r
    bf16 = mybir.dt.bfloat16
    fp8 = mybir.dt.float8e4
    AF = mybir.ActivationFunctionType
    MUL = mybir.AluOpType.mult
    ADD = mybir.AluOpType.add
    DIV = mybir.AluOpType.divide
    DR = mybir.MatmulPerfMode.DoubleRow

    nc = bass.Bass("TRN2", target_bir_lowering=False, debug=False,
                   num_devices=NCORES)

    # ---- DRAM parameters (packed/quantized on host)
    xs = nc.declare_dram_parameter("xs", [2, 128, 2, SPOS], fp8, isOutput=False)
    xss = nc.declare_dram_parameter("xss", [2, 128, 2, SPOS], fp8, isOutput=False)
    sks = nc.declare_dram_parameter("sks", [2, 128, 2, SPOS], fp8, isOutput=False)
    skss = nc.declare_dram_parameter("skss", [2, 128, 2, SPOS], fp8, isOutput=False)
    cos4 = nc.declare_dram_parameter("cos4", [2, 128, SPOS], bf16, isOutput=False)
    sin4 = nc.declare_dram_parameter("sin4", [2, 128, SPOS], bf16, isOutput=False)
    wqh = nc.declare_dram_parameter("wqh", [128, 2, 768], fp8, isOutput=False)
    wql = nc.declare_dram_parameter("wql", [128, 2, 768], fp8, isOutput=False)
    wkh = nc.declare_dram_parameter("wkh", [128, 2, 768], fp8, isOutput=False)
    wkl = nc.declare_dram_parameter("wkl", [128, 2, 768], fp8, isOutput=False)
    wvh = nc.declare_dram_parameter("wvh", [128, 2, 512], fp8, isOutput=False)
    wvl = nc.declare_dram_parameter("wvl", [128, 2, 512], fp8, isOutput=False)
    woh = nc.declare_dram_parameter("woh", [128, 4, 256], bf16, isOutput=False)
    bo = nc.declare_dram_parameter("bo", [1, 256], f32r, isOutput=False)
    onesd = nc.declare_dram_parameter("onesd", [1, 512], f32r, isOutput=False)
    out = nc.declare_dram_parameter("out", [2, 2, 128, SPOS], f32, isOutput=True)

    with tile.TileContext(nc) as tc:
        with ExitStack() as es:
            constp = es.enter_context(tc.tile_pool(name="const", bufs=1))
            inp = es.enter_context(tc.tile_pool(name="inp", bufs=1))
            slabp = es.enter_context(tc.tile_pool(name="slab", bufs=1))
            tmpp = es.enter_context(tc.tile_pool(name="tmp", bufs=3))
            rtp = es.enter_context(tc.tile_pool(name="rtp", bufs=1))
            expp = es.enter_context(tc.tile_pool(name="expp", bufs=6))
            outp = es.enter_context(tc.tile_pool(name="outp", bufs=2))
            psim = es.enter_context(tc.tile_pool(
                name="psim", bufs=int(os.environ.get("K_PSIM", "2")),
                space="PSUM"))
            pwork = es.enter_context(tc.tile_pool(name="pwork", bufs=2, space="PSUM"))
            pav = es.enter_context(tc.tile_pool(name="pav", bufs=2, space="PSUM"))

            # ---- constants (tiles declared here; DMAs emitted in the
            # pipeline section so the prologue's critical path loads first)
            def wtile(shape, dt, name):
                return constp.tile(shape, dt, tag=name, name=name)

            wq_h = wtile([128, 2, 768], fp8, "wqh")
            wq_l = wtile([128, 2, 768], fp8, "wql")
            wk_h = wtile([128, 2, 768], fp8, "wkh")
            wk_l = wtile([128, 2, 768], fp8, "wkl")
            wv_h = wtile([128, 2, 512], fp8, "wvh")
            wv_l = wtile([128, 2, 512], fp8, "wvl")
            wo_t = wtile([128, 4, 256], bf16, "woh")
            bo_r = wtile([1, 256], f32r, "bor")
            onerow = wtile([1, 512], f32r, "onerow")
            ones_e = constp.tile([128, 2, 128], fp8, tag="onese", name="ones_e")
            ones_o = constp.tile([128, 2, 128], fp8, tag="oneso", name="ones_o")

            def dma_startup():
                # interleave weight and nt0-chunk DMAs in exactly the order
                # the first projection units consume them, so the prologue's
                # DVE chain starts as early as possible.
                n0 = slice(0, 512)
                if os.environ.get("K_DMAO", "qxk") == "qxk":
                    seq = ["wq", "x", "cs", "wk", "sk"]
                else:
                    seq = ["wq", "wk", "x", "cs", "sk"]
                for part in seq:
                    if part == "wq":
                        nc.sync.dma_start(out=wq_h[:], in_=wqh[:])
                        nc.sync.dma_start(out=wq_l[:], in_=wql[:])
                    elif part == "wk":
                        nc.sync.dma_start(out=wk_h[:], in_=wkh[:])
                        nc.sync.dma_start(out=wk_l[:], in_=wkl[:])
                    elif part == "x":
                        nc.sync.dma_start(out=x_t[0][:, :, n0], in_=xs[0][:, :, n0])
                        nc.sync.dma_start(out=xl_t[0][:, :, n0], in_=xss[0][:, :, n0])
                    elif part == "cs":
                        nc.sync.dma_start(out=cos_t[0][:, n0], in_=cos4[0][:, n0])
                        nc.sync.dma_start(out=sin_t[0][:, n0], in_=sin4[0][:, n0])
                    else:
                        nc.sync.dma_start(out=sk_t[0][:, :, n0], in_=sks[0][:, :, n0])
                        nc.sync.dma_start(out=skl_t[0][:, :, n0], in_=skss[0][:, :, n0])

            def dma_rest_consts():
                for t, d in ((wv_h, wvh), (wv_l, wvl), (wo_t, woh),
                             (bo_r, bo), (onerow, onesd)):
                    nc.sync.dma_start(out=t[:], in_=d[:])
                nc.gpsimd.memset(ones_e[:], 0.0)
                nc.gpsimd.memset(ones_e[:, :, 0:64], 1.0)
                nc.gpsimd.memset(ones_o[:], 0.0)
                nc.gpsimd.memset(ones_o[:, :, 64:128], 1.0)

            # ---- per-stripe input tiles
            x_t, xl_t, sk_t, skl_t, cos_t, sin_t = {}, {}, {}, {}, {}, {}
            for s in (0, 1):
                x_t[s] = inp.tile([128, 2, SPOS], fp8, tag=f"x{s}", name=f"x{s}")
                xl_t[s] = inp.tile([128, 2, SPOS], fp8, tag=f"xl{s}", name=f"xl{s}")
                sk_t[s] = inp.tile([128, 2, SPOS], fp8, tag=f"sk{s}", name=f"sk{s}")
                skl_t[s] = inp.tile([128, 2, SPOS], fp8, tag=f"skl{s}", name=f"skl{s}")
                cos_t[s] = inp.tile([128, SPOS], bf16, tag=f"cos{s}", name=f"cos{s}")
                sin_t[s] = inp.tile([128, SPOS], bf16, tag=f"sin{s}", name=f"sin{s}")

            def dma_input_chunk(s, nt):
                ntsl = slice(nt * 512, (nt + 1) * 512)
                nc.sync.dma_start(out=x_t[s][:, :, ntsl], in_=xs[s][:, :, ntsl])
                nc.sync.dma_start(out=xl_t[s][:, :, ntsl], in_=xss[s][:, :, ntsl])
                nc.sync.dma_start(out=cos_t[s][:, ntsl], in_=cos4[s][:, ntsl])
                nc.sync.dma_start(out=sin_t[s][:, ntsl], in_=sin4[s][:, ntsl])
                nc.sync.dma_start(out=sk_t[s][:, :, ntsl], in_=sks[s][:, :, ntsl])
                nc.sync.dma_start(out=skl_t[s][:, :, ntsl], in_=skss[s][:, :, ntsl])

            # ---- per-stripe slabs
            q_sl, k_sl, ks_sl, vx_sl, vs_sl, u_sl = {}, {}, {}, {}, {}, {}
            for s in (0, 1):
                for g in (0, 1):
                    q_sl[s, g] = slabp.tile([128, 2, SPOS], fp8, tag=f"q{s}{g}", name=f"q{s}{g}")
                    k_sl[s, g] = slabp.tile([128, 2, SPOS], fp8, tag=f"k{s}{g}", name=f"k{s}{g}")
                    ks_sl[s, g] = slabp.tile([128, 2, SPOS], fp8, tag=f"ks{s}{g}", name=f"ks{s}{g}")
                vx_sl[s] = slabp.tile([128, 16, 768], fp8, tag=f"vx{s}", name=f"vx{s}")
                vs_sl[s] = slabp.tile([128, 16, 768], fp8, tag=f"vs{s}", name=f"vs{s}")
                u_sl[s] = slabp.tile([128, 4, SPOS], bf16, tag=f"u{s}", name=f"u{s}")

            def memset_vslabs(s):
                # deferred: keeps Pool's in-order queue clear for the early
                # rope adds (memsets run in attention's Pool slack instead).
                for vt in (vx_sl[s], vs_sl[s]):
                    zv = vt[:].rearrange("p c (pr t x) -> p (c pr t) x", pr=4, t=3, x=64)
                    nc.gpsimd.memset(zv[:, 1::3, :], 0.0)

            # chunk column offsets in the 768-col weight tiles
            PM = {0: 0, 1: 128}
            PASS = {0: 256, 1: 384}
            ROT = {0: 512, 1: 640}

            # PSUM-eviction copies alternate DVE/ACT per the current phase's
            # pattern so the two eviction-capable engines stay balanced (ACT
            # also owns exp; during exp-free phases it takes more copies).
            ev_state = {"i": 0, "pat": os.environ.get("K_PROPAT", "AVV")}

            def ev_copy(dst, src):
                pat = ev_state["pat"]
                c = pat[ev_state["i"] % len(pat)]
                ev_state["i"] += 1
                if c == "V":
                    nc.vector.tensor_copy(dst, src)
                else:
                    nc.scalar.copy(dst, src)

            def proj_units(s, nts=None, kinds="qkv", groups=(0, 1)):
                """Generator: projections of stripe s in small tile units.
                kinds: "qk" = only q/k/ks units, "v" = only V, "qkv" = both."""
                for nt in (range(NT) if nts is None else nts):
                    ntsl = slice(nt * 512, (nt + 1) * 512)
                    for pi, (wh, wl, rhs_h, rhs_l, dsts) in enumerate((
                        (wq_h, wq_l, x_t[s], xl_t[s], q_sl),
                        (wk_h, wk_l, x_t[s], xl_t[s], k_sl),
                        (wk_h, wk_l, sk_t[s], skl_t[s], ks_sl),
                    ) if "q" in kinds else ()):
                        for g in groups:
                            pm = pwork.tile([128, 512], f32, tag="pw", name="pm")
                            nc.tensor.matmul(pm[:], wh[:, :, PM[g]:PM[g] + 128],
                                             rhs_h[:, :, ntsl], start=True, stop=False,
                                             perf_mode=DR)
                            nc.tensor.matmul(pm[:], wl[:, :, PM[g]:PM[g] + 128],
                                             rhs_l[:, :, ntsl], start=False, stop=True,
                                             perf_mode=DR)
                            pr = pwork.tile([128, 512], f32, tag="pw", name="pr")
                            nc.tensor.matmul(pr[:], wh[:, :, ROT[g]:ROT[g] + 128],
                                             rhs_h[:, :, ntsl], start=True, stop=False,
                                             perf_mode=DR)
                            nc.tensor.matmul(pr[:], wl[:, :, ROT[g]:ROT[g] + 128],
                                             rhs_l[:, :, ntsl], start=False, stop=True,
                                             perf_mode=DR)
                            t1 = tmpp.tile([128, 512], bf16, tag="t1", name="t1")
                            nc.vector.tensor_tensor(out=t1[:], in0=pm[:],
                                                    in1=cos_t[s][:, ntsl], op=MUL)
                            t2 = tmpp.tile([128, 512], bf16, tag="t2", name="t2")
                            nc.vector.tensor_tensor(out=t2[:], in0=pr[:],
                                                    in1=sin_t[s][:, ntsl], op=MUL)
                            nc.gpsimd.tensor_tensor(out=dsts[s, g][:, 0, ntsl],
                                                    in0=t1[:], in1=t2[:], op=ADD)
                            yield
                            pp = pwork.tile([128, 512], f32, tag="pw", name="pp")
                            nc.tensor.matmul(pp[:], wh[:, :, PASS[g]:PASS[g] + 128],
                                             rhs_h[:, :, ntsl], start=True, stop=False,
                                             perf_mode=DR)
                            nc.tensor.matmul(pp[:], wl[:, :, PASS[g]:PASS[g] + 128],
                                             rhs_l[:, :, ntsl], start=False, stop=True,
                                             perf_mode=DR)
                            ev_copy(dsts[s, g][:, 1, ntsl], pp[:])
                            yield
                    # V projections (transposed orientation), 4 pos-chunks/nt/half
                    for hi, (src_h, src_l, vdst) in enumerate((
                        (x_t[s], xl_t[s], vx_sl[s]),
                        (sk_t[s], skl_t[s], vs_sl[s])) if "v" in kinds else ()):
                        for pc in range(4):
                            cidx = nt * 4 + pc
                            psl = slice(nt * 512 + pc * 128, nt * 512 + pc * 128 + 128)
                            pv = pwork.tile([128, 512], f32, tag="pw", name="pv")
                            nc.tensor.matmul(pv[:], src_h[:, :, psl], wv_h[:],
                                             start=True, stop=False, perf_mode=DR)
                            nc.tensor.matmul(pv[:], src_l[:, :, psl], wv_l[:],
                                             start=False, stop=True, perf_mode=DR)
                            vo = vdst[:, cidx, :].rearrange(
                                "p (pr t x) -> p pr t x", pr=4, t=3, x=64)[:, :, ::2, :]
                            vi = pv[:].rearrange("p (pr t x) -> p pr t x",
                                                 pr=4, t=2, x=64)
                            ev_copy(vo, vi)
                            yield

            def outproj_units(s, split_last=False):
                """Output projection of stripe s; bias seeded via K=1 matmul.
                With split_last, the final nt emits both m-halves' bias+kt0/1
                first (overlapping the last attention batch), then kt2/3 and
                the stores — first-halves must all precede second-halves so
                the in-order PE queue never waits on a later flush."""
                for nt in range(NT):
                    ntsl = slice(nt * 512, (nt + 1) * 512)
                    if split_last and nt == NT - 1:
                        fps = []
                        for m in (0, 1):
                            fp = pwork.tile([128, 512], f32, tag="pw", name="fp")
                            nc.tensor.matmul(fp[:], bo_r[:, m * 128:(m + 1) * 128],
                                             onerow[:], start=True, stop=False)
                            for kt in (0, 1):
                                nc.tensor.matmul(fp[:],
                                                 wo_t[:, kt, m * 128:(m + 1) * 128],
                                                 u_sl[s][:, kt, ntsl],
                                                 start=False, stop=False)
                            fps.append(fp)
                            yield
                        for m in (0, 1):
                            fp = fps[m]
                            for kt in (2, 3):
                                nc.tensor.matmul(fp[:],
                                                 wo_t[:, kt, m * 128:(m + 1) * 128],
                                                 u_sl[s][:, kt, ntsl],
                                                 start=False, stop=(kt == 3))
                            osb = outp.tile([128, 512], f32, tag="ot", name="osb")
                            ev_copy(osb[:], fp[:])
                            nc.sync.dma_start(out=out[s, m, :, ntsl], in_=osb[:])
                            yield
                        continue
                    for m in (0, 1):
                        fp = pwork.tile([128, 512], f32, tag="pw", name="fp")
                        nc.tensor.matmul(fp[:], bo_r[:, m * 128:(m + 1) * 128],
                                         onerow[:], start=True, stop=False)
                        for kt in range(4):
                            nc.tensor.matmul(fp[:], wo_t[:, kt, m * 128:(m + 1) * 128],
                                             u_sl[s][:, kt, ntsl],
                                             start=False, stop=(kt == 3))
                        osb = outp.tile([128, 512], f32, tag="ot", name="osb")
                        ev_copy(osb[:], fp[:])
                        nc.sync.dma_start(out=out[s, m, :, ntsl], in_=osb[:])
                        yield

            def emit_sim_window(s, y, h):
                """sim + exp for one window (y, head h); returns exp tile."""
                g, po = h // 4, (h % 4) * 32
                st = psim.tile([128, 4, 256], f32, tag="sim", name="sim")
                for mc in range(4):
                    ksl = (k_sl if mc < 2 else ks_sl)[s, g]
                    lhsT = ksl[po:po + 32, :,
                               y * 256 + (mc % 2) * 128:
                               y * 256 + (mc % 2) * 128 + 128]
                    rhs = q_sl[s, g][po:po + 32, :, y * 256:y * 256 + 256]
                    nc.tensor.matmul(st[:, mc, :], lhsT, rhs,
                                     start=True, stop=True, perf_mode=DR,
                                     tile_position=(po, 0))
                et = expp.tile([128, 4, 256], fp8, tag="exp", name="exp")
                nc.scalar.activation(et[:], st[:], AF.Exp, scale=float(DH) ** -0.5)
                return et

            def emit_av_pair(s, y, jb, ets, avt, dt, bi):
                """AV + denominator matmuls for pair (y, jb + bi) only."""
                j = jb + bi
                # windows (y, 2j) -> rows 0:64 and (y, 2j+1) -> rows 64:128
                # via zero-padded lhsT; 4 matmuls accumulate one region.
                step = 0
                for o in (0, 1):
                    pb = j * 192 + o * 64
                    et = ets[2 * bi + o]
                    for mc, vsl in ((0, vx_sl[s]), (1, vs_sl[s])):
                        nc.tensor.matmul(avt[:, bi * 256:bi * 256 + 256],
                                         vsl[:, 2 * y:2 * y + 2, pb:pb + 128],
                                         et[:, 2 * mc:2 * mc + 2, :],
                                         start=(step == 0), stop=(step == 3),
                                         perf_mode=DR)
                        step += 1
                step = 0
                for o, ot in ((0, ones_e), (1, ones_o)):
                    et = ets[2 * bi + o]
                    for mc in (0, 1):
                        nc.tensor.matmul(dt[:, bi * 256:bi * 256 + 256],
                                         ot[:],
                                         et[:, 2 * mc:2 * mc + 2, :],
                                         start=(step == 0), stop=(step == 3),
                                         perf_mode=DR)
                        step += 1

            def emit_av_norm(s, y, jb, avt, dt):
                # HW: TensorTensor may read at most ONE input from PSUM, so
                # the denominator reciprocal goes through SBUF, then DVE
                # multiplies avt (PSUM) by it.
                rt = rtp.tile([128, 512], f32, tag="rt", name="rt")
                with nc.allow_low_precision(reason="softmax reciprocal"):
                    nc.vector.reciprocal(rt[:], dt[:])
                nc.vector.tensor_tensor(
                    out=u_sl[s][:, jb:jb + 2, y * 256:y * 256 + 256],
                    in0=avt[:], in1=rt[:], op=MUL)

            def attn_stream(batches, filler, pull_plan, on_y_done=None,
                            on_flush=None):
                """Single interleaved attention stream over (s, y, jb)
                batches. The previous batch's AV work is staged across this
                batch's window slots (pair-0 mms at w1, pair-1 mms at w2,
                normalize at w3) so no PE burst starves the exp chain;
                pull_plan[i] filler units interleave after w2/w3."""
                prev = None
                def flush_stage(stage):
                    nonlocal prev
                    if prev is None:
                        return
                    ps, py, pjb, pets, pav_t = prev
                    if stage == 1:
                        emit_av_pair(ps, py, pjb, pets, *pav_t, 0)
                    elif stage == 2:
                        emit_av_pair(ps, py, pjb, pets, *pav_t, 1)
                    else:
                        emit_av_norm(ps, py, pjb, *pav_t)
                        if on_y_done is not None:
                            on_y_done(ps, py, pjb)
                        prev = None
                        if on_flush is not None:
                            on_flush()
                def flush_all():
                    flush_stage(1); flush_stage(2); flush_stage(3)
                # per-phase eviction pattern: ACT idles between exps where
                # the projection pipeline is the pacer (span start, stripe-1
                # join), so the copy share shifts by batch index.
                base_pat = ev_state["pat"]
                sched = []
                for kv in os.environ.get("K_PATPLAN", "0:AV,4:-").split(","):
                    k, v = kv.split(":")
                    sched.append((int(k), base_pat if v == "-" else v))
                sched.sort()
                for i, (s, y, jb) in enumerate(batches):
                    for bi, p in sched:
                        if i >= bi:
                            ev_state["pat"] = p
                    np_ = pull_plan[i] if i < len(pull_plan) else 0
                    pulled = 0
                    ets = []
                    for wi in range(4):
                        ets.append(emit_sim_window(s, y, 2 * jb + wi))
                        if wi in (1, 2):
                            flush_stage(wi)
                        if wi in (2, 3):
                            take = (np_ * (wi - 1)) // 2 - pulled
                            for _ in range(take):
                                next(filler, None)
                            pulled += take
                        if wi == 3:
                            flush_stage(3)
                    avt = pav.tile([128, 512], f32, tag="av", name="avt")
                    dt = pav.tile([128, 512], f32, tag="av", name="dt")
                    prev = (s, y, jb, ets, (avt, dt))
                flush_all()
                for _ in filler:
                    pass

            def drain(gen):
                for _ in gen:
                    pass

            # ---- pipeline
            import itertools
            dma_startup()
            dma_rest_consts()
            for nt in range(1, NT):
                dma_input_chunk(0, nt)
            npro = int(os.environ.get("K_PRO", "1"))
            ev_state["pat"] = os.environ.get("K_PROPAT", "AVV")
            pmode = os.environ.get("K_PROKINDS", "gfirst")
            if pmode == "g0":
                # minimal prologue: only the g0 q/k/ks units of nt0 gate the
                # first attention batch (heads 0-3); everything else fills.
                drain(proj_units(0, (0,), kinds="qk", groups=(0,)))
            elif pmode == "gfirst":
                # batch 0 (heads 0-3) needs only the g0 q/k/ks units: emit
                # all of g0 before any g1 so its sims unblock ~2.6us earlier.
                drain(proj_units(0, (0,), kinds="qk", groups=(0,)))
                drain(proj_units(0, (0,), kinds="qk", groups=(1,)))
                drain(proj_units(0, (0,), kinds="v"))
                for pnt in range(1, npro):
                    drain(proj_units(0, (pnt,)))
            else:
                drain(proj_units(0, tuple(range(npro)), kinds=pmode))
            memset_vslabs(0)
            for nt in range(NT):
                dma_input_chunk(1, nt)
            memset_vslabs(1)
            ev_state["pat"] = os.environ.get("K_EVPAT", "VAVVAV")

            # one interleaved batch stream: stripe 1 joins once its
            # projections (pulled as filler) are far enough along.
            lead = int(os.environ.get("K_LEAD", "16"))     # s0-only batches
            b0 = [(0, y, jb) for y in range(YW) for jb in (0, 2)]
            jsw = int(os.environ.get("K_JSWZ", "2"))
            if jsw:
                # first `jsw` y-windows run jb=0 (g0 heads) before any jb=2
                # (g1) batch, hiding the prologue's g1 unit latency.
                head = ([(0, y, 0) for y in range(jsw)]
                        + [(0, y, 2) for y in range(jsw)])
                b0 = head + b0[2 * jsw:]
            b1 = [(1, y, jb) for y in range(YW) for jb in (0, 2)]
            batches = list(b0[:lead])
            for k in range(lead, 16):
                batches.append(b1[k - lead])
                batches.append(b0[k])
            batches.extend(b1[16 - lead:])
            assert len(batches) == 32

            if pmode == "g0":
                filler = itertools.chain(
                    proj_units(0, (0,), kinds="qk", groups=(1,)),
                    proj_units(0, (0,), kinds="v"),
                    proj_units(0, tuple(range(1, 4))),
                    proj_units(1, tuple(range(NT))))
                nfill = 6 + 8 + 60 + 80
            elif pmode == "qk":
                filler = itertools.chain(
                    proj_units(0, tuple(range(npro)), kinds="v"),
                    proj_units(0, tuple(range(npro, 4))),
                    proj_units(1, tuple(range(NT))))
                nfill = npro * 8 + (4 - npro) * 20 + 80
            else:
                def gfirst_units(s, nts):
                    # per nt: all g0 q/k/ks units, then g1, then V — so each
                    # jb=0 batch unblocks before its stripe's g1 work lands.
                    for nt in nts:
                        yield from proj_units(s, (nt,), kinds="qk", groups=(0,))
                        yield from proj_units(s, (nt,), kinds="qk", groups=(1,))
                        yield from proj_units(s, (nt,), kinds="v")
                if os.environ.get("K_FGF", "1") == "1":
                    filler = itertools.chain(
                        gfirst_units(0, tuple(range(npro, 4))),
                        gfirst_units(1, tuple(range(NT))))
                else:
                    filler = itertools.chain(
                        proj_units(0, tuple(range(npro, 4))),
                        proj_units(1, tuple(range(NT))))
                nfill = (4 - npro) * 20 + 80
            n0 = int(os.environ.get("K_N0", "0"))          # extra-heavy start
            p0 = int(os.environ.get("K_P0", "7"))
            early = int(os.environ.get("K_EARLY", "16"))   # heavy-pull batches
            ep = int(os.environ.get("K_EPULL", "5"))
            rn = int(os.environ.get("K_RN", "13"))         # spread remainder
            rest = max(0, nfill - n0 * p0 - early * ep)
            pull_plan = ([p0] * n0 + [ep] * early
                         + [rest // rn + (1 if i < rest % rn else 0)
                            for i in range(rn)]
                         + [0] * 32)

            ops = {0: outproj_units(0),
                   1: outproj_units(1, split_last=os.environ.get("K_TSPLIT", "0") == "1")}
            pend = []
            def y_done(s, y, jb):
                if (s == 1 and y == YW - 1 and jb == 0
                        and os.environ.get("K_TSPLIT", "0") == "1"):
                    # last-nt outproj first-halves: only need u kt0/1, which
                    # this flush just produced — overlap the final batch.
                    next(ops[1], None)
                    next(ops[1], None)
                elif jb == 2 and y % 2 == 1:
                    pend.extend([s, s])
            def on_flush():
                # spread outproj units one per flush so their PE bursts don't
                # delay the next window's sims.
                if pend:
                    next(ops[pend.pop(0)], None)
            attn_stream(batches, filler, pull_plan, on_y_done=y_done,
                        on_flush=on_flush)
            drain(ops[0])
            drain(ops[1])

    _split_excess_waits(nc)
    return nc


def _split_excess_waits(nc, max_waits=1):
    """walrus accepts one sync-wait command per instruction; hoist excess
    waits onto same-engine NoOps inserted just before."""
    import bass_rust
    import concourse.mybir as mybir
    n_added = 0
    for f in nc.m.functions:
        for bb in f.blocks:
            insts = list(bb.instructions)
            new = []
            dirty = False
            for inst in insts:
                si = inst.sync_info
                if si is not None and len(si.on_wait) > max_waits:
                    waits = list(si.on_wait)
                    for wt in waits[:-max_waits]:
                        nop = mybir.InstNoOp(name=f"{inst.name}-ws{n_added}",
                                             ins=[], outs=[])
                        nop.engine = inst.engine
                        nop.sync_info = bass_rust.SyncInfo(on_wait=[wt], on_update=[])
                        new.append(nop)
                        n_added += 1
                    inst.sync_info = bass_rust.SyncInfo(
                        on_wait=waits[-max_waits:], on_update=list(si.on_update))
                    dirty = True
                new.append(inst)
            if dirty:
                bb.instructions = new
    return n_added


def _window_major(a):
    """[..., 16, 128] spatial block -> [..., 2048] window-major positions."""
    lead = a.shape[:-2]
    return (a.reshape(*lead, WIN, YW, WIN)
             .swapaxes(-3, -2)
             .reshape(*lead, SPOS))


def _pack_kt(a):
    """[256, N] -> [128, 2, N] (channel c = kt*128 + p)."""
    n = a.shape[-1]
    return np.ascontiguousarray(a.reshape(2, 128, n).transpose(1, 0, 2))


def _rot_weights(Wm):
    R = np.zeros_like(Wm)
    for h in range(HEADS):
        b0 = h * DH
        for i in range(DRR // 2):
            R[b0 + 2 * i] = -Wm[b0 + 2 * i + 1]
            R[b0 + 2 * i + 1] = Wm[b0 + 2 * i]
    return R


def _qk_weight(Wm):
    """[512, 256] -> [768, 256] rows ordered (pm-g0, pm-g1, pass-g0, pass-g1,
    rot-g0, rot-g1), each chunk of 128 = 4 heads x 32 channels."""
    R = _rot_weights(Wm)
    rows = np.empty((768, C), np.float32)
    for g in (0, 1):
        for p in range(128):
            h, d = 4 * g + p // 32, p % 32
            rows[g * 128 + p] = Wm[h * DH + d]              # pm (rope dims)
            rows[256 + g * 128 + p] = Wm[h * DH + 32 + d]   # pass
            rows[512 + g * 128 + p] = R[h * DH + d]         # rot
    return rows


def _make_core_inputs(x, skip, time_emb, sin, cos, Wq, Wkv, Wout, bout):
    import ml_dtypes
    f8 = ml_dtypes.float8_e4m3
    bfd = ml_dtypes.bfloat16

    def split_resid(rows):  # [M, 256] f32 -> hi/lo [128, 2, M] fp8
        hi = np.asarray(rows, f8).astype(np.float32)
        lo = SC * (np.asarray(rows, np.float32) - hi)
        return (np.asarray(_pack_kt(hi.T), f8),
                np.asarray(_pack_kt(np.asarray(lo, f8).astype(np.float32).T), f8))

    Wk_, Wv_ = Wkv[:512], Wkv[512:]
    wqh_a, wql_a = split_resid(_qk_weight(Wq))
    wkh_a, wkl_a = split_resid(_qk_weight(Wk_))
    wvh_a, wvl_a = split_resid(Wv_)      # natural col order h*64+d

    # wo rows: u-channel (p, kt): h = 2*kt + p//64, d = p%64
    wo_rows = np.empty((128, 4, C), np.float32)
    for p in range(128):
        for kt in range(4):
            h, d = 2 * kt + p // 64, p % 64
            wo_rows[p, kt] = Wout[:, h * DH + d]
    woh_a = np.asarray(wo_rows, bfd)
    bo_a = bout.reshape(1, C).astype(np.float32).copy()

    in_maps = []
    for core in range(NCORES):
        b = core // 4
        xbs = [2 * (core % 4), 2 * (core % 4) + 1]
        xs_c = np.empty((2, 128, 2, SPOS), f8)
        xss_c = np.empty((2, 128, 2, SPOS), f8)
        sk_c = np.empty((2, 128, 2, SPOS), f8)
        skss_c = np.empty((2, 128, 2, SPOS), f8)
        cos_c = np.empty((2, 128, SPOS), bfd)
        sin_c = np.empty((2, 128, SPOS), bfd)
        for si, xb in enumerate(xbs):
            rs = slice(xb * WIN, (xb + 1) * WIN)
            xf = _window_major((x[b, :, rs, :] + time_emb[b][:, None, None])
                               .astype(np.float32))
            skf = _window_major(skip[b, :, rs, :].astype(np.float32))
            xs_c[si] = np.asarray(_pack_kt(xf), f8)
            xss_c[si] = np.asarray(_pack_kt(xf / SC), f8)
            sk_c[si] = np.asarray(_pack_kt(skf), f8)
            skss_c[si] = np.asarray(_pack_kt(skf / SC), f8)
            cw = _window_major(cos[rs].transpose(2, 0, 1))   # [32, 2048]
            sw = _window_major(sin[rs].transpose(2, 0, 1))
            cos_c[si] = np.asarray(np.tile(cw, (4, 1)), bfd)
            sin_c[si] = np.asarray(np.tile(sw, (4, 1)), bfd)
        in_maps.append({
            "xs": xs_c, "xss": xss_c, "sks": sk_c, "skss": skss_c,
            "cos4": cos_c, "sin4": sin_c,
            "wqh": wqh_a, "wql": wql_a, "wkh": wkh_a, "wkl": wkl_a,
            "wvh": wvh_a, "wvl": wvl_a, "woh": woh_a, "bo": bo_a,
            "onesd": np.ones((1, 512), np.float32),
        })
    return in_maps


def _assemble(results):
    out_full = np.empty((B, C, H, W), np.float32)
    for core in range(NCORES):
        b = core // 4
        xbs = [2 * (core % 4), 2 * (core % 4) + 1]
        o = results[core]["out"]          # [2, 2, 128, 2048] window-major
        for si, xb in enumerate(xbs):
            ch = o[si].reshape(C, YW, WIN, WIN)
            blk = ch.swapaxes(1, 2).reshape(C, WIN, W)
            out_full[b, :, xb * WIN:(xb + 1) * WIN, :] = blk
    return out_full


def get_nc():
    if "nc" not in _CACHE:
        _CACHE["nc"] = _build()
    return _CACHE["nc"]


def kernel(x, skip, time_emb, sin, cos, Wq, Wkv, Wout, bout):
    from concourse.bass_utils import run_bass_kernel_spmd
    args = [np.asarray(a, dtype=np.float32) for a in
            (x, skip, time_emb, sin, cos, Wq, Wkv, Wout, bout)]
    nc = get_nc()
    in_maps = _make_core_inputs(*args)
    res = run_bass_kernel_spmd(nc, in_maps, list(range(NCORES)), trace=False)
    return _assemble(res.results)

